# revision 13
# baseline (speedup 1.0000x reference)
"""Fused linear + cross-entropy loss (BaseChunkLoss) on 8 trn2 NeuronCores.

Strategy (per the sharding hint: tensor-parallel over vocab):
  - head_weight is sharded 8 ways over the vocab dim: each core handles the
    FULL 8192 tokens x a 4000-entry vocab slice and produces the partial
    sum_{v in shard} exp(logit[t, v]) for every token.  The cross-device
    logsumexp reduction (sum of the 8 partials, then log) plus the weighted
    mean happen on host, standing in for the wrapper's all_reduce.
  - This puts each core's HBM traffic at ~117 MB (full hidden 67 MB + W
    slice 33 MB + target-row gather 17 MB) -- under the fp8 PE roofline --
    instead of the 290 MB/core a token-sharded design pays to stream the
    whole 262 MB weight through every core.
  - The W slice is cast to fp8 once and stays SBUF-resident; hidden^T
    streams through in 1024-token chunks, cast to fp8 on the fly.

Device kernel layout: tokens on PSUM partitions, vocab on the free dim.
  stationary lhsT = hidden^T tile [128 d x 2 x 128 tok]   (fp8, DoubleRow)
  moving rhs      = weight^T tile [128 d x 2 x 500 vocab]
  psum [128 tok x 500 vocab] fp32, accumulated over D=2048 in 8 matmuls.
Weights are pre-scaled by 64 on-chip for e4m3 range and descaled during the
bias add.  Per 1000-wide vocab group (2 psum banks, 4 groups in flight):
DVE does (psum/64 + bias) in place, ACT computes exp with a fused free-dim
row-sum accumulator into s_cols.  The target logit is computed exactly in
f32 as a DVE rowdot of the core's 1024-token hidden slice against the
host-gathered W[labels] rows; host adds bias[labels].

Host-side input prep is layout-only (transpose/slice/gather of rows); all
FLOPs over hidden/weights happen on device inside the measured kernel.
"""
import numpy as np
from contextlib import ExitStack

from concourse import bacc, mybir, tile
from concourse.bass_utils import run_bass_kernel_spmd

F32 = mybir.dt.float32
FP8 = mybir.dt.float8e4
Alu = mybir.AluOpType
Act = mybir.ActivationFunctionType

N_CORES = 8
N_TOK = 8192
D = 2048
V = 32000
P = 128

VSH = V // N_CORES      # 4000 vocab entries per core
TC = N_TOK // N_CORES   # 1024 tokens per core (for the exact tgt rowdot)
KP2 = D // (2 * P)      # 8 DoubleRow contraction steps of K=256
BANK = 500              # vocab columns per psum bank (<= 512 fp32)
BPG = 2                 # banks per vocab group
GV = BPG * BANK         # 1000 vocab per group
NG = VSH // GV          # 4 groups
CHT = 1024              # tokens per streamed hidden chunk
NCH = N_TOK // CHT      # 8 chunks
MBC = CHT // P          # 8 token blocks per chunk
MBT = N_TOK // P        # 64 token blocks total
HSP = 512               # tokens per hidden DMA piece
DHALF = D // 2          # rowdot split for SBUF economy

W_SCALE = 64.0          # fp8 weight pre-scale (e4m3 range)

_DBG_LABELS = {}


def _lab(inst, label):
    try:
        _DBG_LABELS[inst.name] = label
    except Exception:
        pass
    return inst


def _build():
    nc = bacc.Bacc("TRN2", target_bir_lowering=False, debug=False)
    h_d = nc.declare_dram_parameter("h", [D, N_TOK], F32, isOutput=False)
    W_d = nc.declare_dram_parameter("W", [D, VSH], F32, isOutput=False)
    bias_d = nc.declare_dram_parameter("bias", [VSH], F32, isOutput=False)
    hn_d = nc.declare_dram_parameter("hn", [TC, D], F32, isOutput=False)
    wg_d = nc.declare_dram_parameter("wg", [TC, D], F32, isOutput=False)
    s_out = nc.declare_dram_parameter("s_out", [P, MBT * NG], F32, isOutput=True)
    t_out = nc.declare_dram_parameter("t_out", [P, TC // P * 2], F32, isOutput=True)

    h_r2 = h_d[:].rearrange("(kp j ki) t -> kp ki j t", ki=P, j=2)
    W_r2 = W_d[:].rearrange("(kp j ki) v -> kp ki j v", ki=P, j=2)

    with tile.TileContext(nc) as tc, ExitStack() as ctx:
        wpool = ctx.enter_context(tc.tile_pool(name="w", bufs=1))
        wstage = ctx.enter_context(tc.tile_pool(name="wstage", bufs=2))
        hpool = ctx.enter_context(tc.tile_pool(name="hT", bufs=4))
        hstage = ctx.enter_context(tc.tile_pool(name="hstage", bufs=2))
        bpool = ctx.enter_context(tc.tile_pool(name="bias", bufs=1))
        gpool = ctx.enter_context(tc.tile_pool(name="gath", bufs=2))
        djunk = ctx.enter_context(tc.tile_pool(name="djunk", bufs=1))
        ejunk = ctx.enter_context(tc.tile_pool(name="ejunk", bufs=2))
        pspool = ctx.enter_context(tc.tile_pool(name="ps", bufs=4, space="PSUM"))
        acc = ctx.enter_context(tc.tile_pool(name="acc", bufs=1))

        s_cols = acc.tile([P, MBT * NG], F32, tag="scols")
        t_cols = acc.tile([P, TC // P * 2], F32, tag="tcols")

        bb = bpool.tile([P, VSH], F32, tag="bias")

        def stage_bias(g):
            v0 = g * GV
            nc.sync.dma_start(
                bb[:, v0:v0 + GV], bias_d[v0:v0 + GV].partition_broadcast(P))

        h_tiles = [None] * NCH

        def stage_h(c):
            # piece order s-outer/kp-inner so early token blocks complete
            # (and unblock their matmuls) before the whole chunk lands
            hc = hpool.tile([P, KP2, 2, CHT], FP8, tag="hT")
            for s in range(CHT // HSP):
                for kp in range(KP2):
                    t0 = c * CHT + s * HSP
                    st = hstage.tile([P, 2, HSP], F32, tag="hstage")
                    _lab(nc.sync.dma_start(st[:], h_r2[kp][:, :, t0:t0 + HSP]),
                         f"dma_h c{c} s{s} kp{kp}")
                    _lab(nc.gpsimd.tensor_copy(
                        hc[:, kp, :, s * HSP:(s + 1) * HSP], st[:]),
                         f"cast_h c{c} s{s} kp{kp}")
            h_tiles[c] = hc

        wv = wpool.tile([P, KP2, 2, VSH], FP8, tag="w")

        def stage_w(g):
            v0 = g * GV
            for kp in range(KP2):
                ws = wstage.tile([P, 2, GV], F32, tag="wstage")
                _lab(nc.sync.dma_start(ws[:], W_r2[kp][:, :, v0:v0 + GV]),
                     f"dma_w g{g} kp{kp}")
                _lab(nc.gpsimd.tensor_scalar_mul(
                    wv[:, kp, :, v0:v0 + GV], ws[:], W_SCALE),
                     f"cast_w g{g} kp{kp}")

        def compute(c, mm, g):
            m = c * MBC + mm
            pt = pspool.tile([P, BPG, 512], F32, tag="ps")
            lhsT = h_tiles[c][:, :, :, mm * P:(mm + 1) * P]
            # kp descending: the first matmul issued depends on the LAST
            # W/h piece to arrive, so a tile's 16 matmuls run as one warm
            # PE stretch instead of trickling at cold pstate per piece
            for ki, kp in enumerate(reversed(range(KP2))):
                for bk in range(BPG):
                    _lab(nc.tensor.matmul(
                        pt[:, bk, 0:BANK], lhsT[:, kp],
                        wv[:, kp, :, g * GV + bk * BANK:g * GV + (bk + 1) * BANK],
                        start=(ki == 0), stop=(ki == KP2 - 1),
                        perf_mode=mybir.MatmulPerfMode.DoubleRow,
                    ), f"mm c{c} m{mm} g{g} kp{kp} bk{bk}")
            psl = pt[:, 0:BPG, 0:BANK]
            bbv = bb[:, g * GV:(g + 1) * GV].rearrange("p (b c) -> p b c", c=BANK)
            _lab(nc.vector.scalar_tensor_tensor(
                psl, psl, 1.0 / W_SCALE, bbv, op0=Alu.mult, op1=Alu.add),
                 f"bias c{c} m{mm} g{g}")
            et = ejunk.tile([P, BPG, BANK], F32, tag="ejunk")
            col = m * NG + g
            _lab(nc.scalar.activation(
                et[:], psl, Act.Exp, accum_out=s_cols[:, col:col + 1]),
                 f"exp c{c} m{mm} g{g}")

        def rowdot(r):
            # exact f32 target logit for token block r of this core's slice
            for hh in range(2):
                hg = gpool.tile([P, DHALF], F32, tag="hg")
                nc.sync.dma_start(
                    hg[:], hn_d[r * P:(r + 1) * P, hh * DHALF:(hh + 1) * DHALF])
                wgt = gpool.tile([P, DHALF], F32, tag="wgt")
                nc.sync.dma_start(
                    wgt[:], wg_d[r * P:(r + 1) * P, hh * DHALF:(hh + 1) * DHALF])
                dj = djunk.tile([P, DHALF], F32, tag="djunk")
                nc.vector.tensor_tensor_reduce(
                    dj[:], hg[:], wgt[:], 1.0, 0.0, op0=Alu.mult, op1=Alu.add,
                    accum_out=t_cols[:, r * 2 + hh:r * 2 + hh + 1])

        # -- prologue: interleave W groups, bias slices and h chunks on the
        # DMA queue; traverse compute in the same order the data arrives so
        # the in-order PE stream never waits on a far-future transfer --
        stage_w(0)
        stage_bias(0)
        stage_h(0)
        stage_h(1)
        stage_w(1)
        stage_bias(1)
        stage_h(2)
        stage_w(2)
        stage_bias(2)
        stage_w(3)
        stage_bias(3)

        # coarse (chunk, group) work units: fragmenting finer than this makes
        # the PE run cold (pstate ramp) during the trickle phase
        for c, g in (
            (0, 0), (1, 0), (0, 1), (1, 1), (2, 0), (2, 1),
            (0, 2), (1, 2), (2, 2), (0, 3), (1, 3), (2, 3),
        ):
            for mm in range(MBC):
                compute(c, mm, g)

        # steady state: prefetch two chunks ahead, compute chunk c
        stage_h(3)
        stage_h(4)
        for c in range(3, NCH):
            if c + 2 < NCH:
                stage_h(c + 2)
            for mm in range(MBC):
                for g in range(NG):
                    compute(c, mm, g)
            # spread the 8 exact-tgt rowdots over mid-stream chunks
            if 3 <= c <= 6:
                rowdot(2 * (c - 3))
                rowdot(2 * (c - 3) + 1)

        nc.sync.dma_start(s_out[:], s_cols[:])
        nc.sync.dma_start(t_out[:], t_cols[:])

    nc.compile()
    return nc


_NC_CACHE = {}


def _get_program():
    if "v" not in _NC_CACHE:
        _NC_CACHE["v"] = _build()
    return _NC_CACHE["v"]


def kernel(hidden_states, head_weight, head_bias, loss_weight, labels,
           chunk_size=None, **_unused):
    hidden = np.asarray(hidden_states, dtype=np.float32)
    W = np.asarray(head_weight, dtype=np.float32)
    bias = np.asarray(head_bias, dtype=np.float32)
    lw = np.asarray(loss_weight, dtype=np.float32)
    labels = np.asarray(labels).astype(np.int64)

    assert hidden.shape == (N_TOK, D) and W.shape == (V, D)

    nc = _get_program()
    Wt = np.ascontiguousarray(W.T)                 # [D, V]
    ht = np.ascontiguousarray(hidden.T)            # [D, N]
    Wg = W[labels]                                 # gathered rows [N, D]
    in_maps = []
    for c in range(N_CORES):
        vsl = slice(c * VSH, (c + 1) * VSH)
        tsl = slice(c * TC, (c + 1) * TC)
        in_maps.append(dict(
            h=ht,
            W=np.ascontiguousarray(Wt[:, vsl]),
            bias=np.ascontiguousarray(bias[vsl]),
            hn=np.ascontiguousarray(hidden[tsl]),
            wg=np.ascontiguousarray(Wg[tsl]),
        ))
    res = run_bass_kernel_spmd(nc, in_maps, list(range(N_CORES)))

    # unshard + host-side scalar combine (the "all_reduce" of the hint):
    # sum the 8 per-core vocab-shard partials of sum_v exp(logit) per token
    s = np.zeros(N_TOK, dtype=np.float64)
    for r in res.results:
        sc = r["s_out"].astype(np.float64).reshape(P, MBT, NG).sum(axis=2)
        s += sc.T.reshape(N_TOK)
    # exact f32 target dot h . W[label] (+ bias) per token
    tgt = np.concatenate([
        r["t_out"].astype(np.float64).reshape(P, TC // P, 2).sum(axis=2)
        .T.reshape(TC)
        for r in res.results])
    tgt = tgt + bias[labels].astype(np.float64)
    lse = np.log(s)
    nll = lse - tgt
    w64 = lw.astype(np.float64)
    loss = (w64 * nll).sum() / max(w64.sum(), 1.0)
    return np.float32(loss)


# revision 16
# speedup vs baseline: 1.0136x; 1.0136x over previous
"""Fused linear + cross-entropy loss (BaseChunkLoss) on 8 trn2 NeuronCores.

Strategy (per the sharding hint: tensor-parallel over vocab):
  - head_weight is sharded 8 ways over the vocab dim: each core handles the
    FULL 8192 tokens x a 4000-entry vocab slice and produces the partial
    sum_{v in shard} exp(logit[t, v]) for every token.  The cross-device
    logsumexp reduction (sum of the 8 partials, then log) plus the weighted
    mean happen on host, standing in for the wrapper's all_reduce.
  - This puts each core's HBM traffic at ~117 MB (full hidden 67 MB + W
    slice 33 MB + target-row gather 17 MB) -- under the fp8 PE roofline --
    instead of the 290 MB/core a token-sharded design pays to stream the
    whole 262 MB weight through every core.
  - The W slice is cast to fp8 once and stays SBUF-resident; hidden^T
    streams through in 1024-token chunks, cast to fp8 on the fly.

Device kernel layout: tokens on PSUM partitions, vocab on the free dim.
  stationary lhsT = hidden^T tile [128 d x 2 x 128 tok]   (fp8, DoubleRow)
  moving rhs      = weight^T tile [128 d x 2 x 500 vocab]
  psum [128 tok x 500 vocab] fp32, accumulated over D=2048 in 8 matmuls.
Weights are pre-scaled by 64 on-chip for e4m3 range and descaled during the
bias add.  Per 1000-wide vocab group (2 psum banks, 4 groups in flight):
DVE does (psum/64 + bias) in place, ACT computes exp with a fused free-dim
row-sum accumulator into s_cols.  The target logit is computed exactly in
f32 as a DVE rowdot of the core's 1024-token hidden slice against the
host-gathered W[labels] rows; host adds bias[labels].

Host-side input prep is layout-only (transpose/slice/gather of rows); all
FLOPs over hidden/weights happen on device inside the measured kernel.
"""
import numpy as np
from contextlib import ExitStack

from concourse import bacc, mybir, tile
from concourse.bass_utils import run_bass_kernel_spmd

F32 = mybir.dt.float32
FP8 = mybir.dt.float8e4
Alu = mybir.AluOpType
Act = mybir.ActivationFunctionType

N_CORES = 8
N_TOK = 8192
D = 2048
V = 32000
P = 128

VSH = V // N_CORES      # 4000 vocab entries per core
TC = N_TOK // N_CORES   # 1024 tokens per core (for the exact tgt rowdot)
KP2 = D // (2 * P)      # 8 DoubleRow contraction steps of K=256
BANK = 500              # vocab columns per psum bank (<= 512 fp32)
BPG = 2                 # banks per vocab group
GV = BPG * BANK         # 1000 vocab per group
NG = VSH // GV          # 4 groups
CHT = 1024              # tokens per streamed hidden chunk
NCH = N_TOK // CHT      # 8 chunks
MBC = CHT // P          # 8 token blocks per chunk
MBT = N_TOK // P        # 64 token blocks total
HSP = 512               # tokens per hidden DMA piece
DHALF = D // 2          # rowdot split for SBUF economy

W_SCALE = 64.0          # fp8 weight pre-scale (e4m3 range)

# schedule knobs (tuned empirically against TimelineSim)
KP_REV_MM = False       # issue matmuls kp-descending
KP_REV_W = False        # stage W pieces kp-descending
KP_REV_H = False        # stage h pieces kp-descending
H_BUFS = 3              # hT chunk double-buffer depth
PREFETCH2 = False       # prefetch two h chunks ahead
# PE pstate warmup: dummy matmuls keep the PE "busy" across DMA-wait gaps so
# real matmuls always run at full clock (the cost model halves the clock for
# ~3us after every idle->busy transition).  N_WARM0 dummies run before the
# first real tile; GAP_DUMMIES[i] dummies run after prologue unit i.
N_WARM0 = 460
GAP_DUMMIES = {}

_DBG_LABELS = {}


def _lab(inst, label):
    try:
        _DBG_LABELS[inst.name] = label
    except Exception:
        pass
    return inst


def _build():
    nc = bacc.Bacc("TRN2", target_bir_lowering=False, debug=False)
    h_d = nc.declare_dram_parameter("h", [D, N_TOK], F32, isOutput=False)
    W_d = nc.declare_dram_parameter("W", [D, VSH], F32, isOutput=False)
    bias_d = nc.declare_dram_parameter("bias", [VSH], F32, isOutput=False)
    hn_d = nc.declare_dram_parameter("hn", [TC, D], F32, isOutput=False)
    wg_d = nc.declare_dram_parameter("wg", [TC, D], F32, isOutput=False)
    s_out = nc.declare_dram_parameter("s_out", [P, MBT * NG], F32, isOutput=True)
    t_out = nc.declare_dram_parameter("t_out", [P, TC // P * 2], F32, isOutput=True)

    h_r2 = h_d[:].rearrange("(kp j ki) t -> kp ki j t", ki=P, j=2)
    W_r2 = W_d[:].rearrange("(kp j ki) v -> kp ki j v", ki=P, j=2)

    with tile.TileContext(nc) as tc, ExitStack() as ctx:
        wpool = ctx.enter_context(tc.tile_pool(name="w", bufs=1))
        wstage = ctx.enter_context(tc.tile_pool(name="wstage", bufs=2))
        hpool = ctx.enter_context(tc.tile_pool(name="hT", bufs=H_BUFS))
        hstage = ctx.enter_context(tc.tile_pool(name="hstage", bufs=2))
        bpool = ctx.enter_context(tc.tile_pool(name="bias", bufs=1))
        gpool = ctx.enter_context(tc.tile_pool(name="gath", bufs=2))
        djunk = ctx.enter_context(tc.tile_pool(name="djunk", bufs=1))
        ejunk = ctx.enter_context(tc.tile_pool(name="ejunk", bufs=2))
        pspool = ctx.enter_context(tc.tile_pool(name="ps", bufs=3, space="PSUM"))
        psjpool = ctx.enter_context(tc.tile_pool(name="psj", bufs=1, space="PSUM"))
        acc = ctx.enter_context(tc.tile_pool(name="acc", bufs=1))

        s_cols = acc.tile([P, MBT * NG], F32, tag="scols")
        t_cols = acc.tile([P, TC // P * 2], F32, tag="tcols")

        # junk operands + psum bank for PE warmup matmuls
        lhs_j = acc.tile([P, 2, P], FP8, tag="lhsj")
        rhs_j = acc.tile([P, 2, 512], FP8, tag="rhsj")
        nc.gpsimd.memset(lhs_j[:], 0.0)
        nc.gpsimd.memset(rhs_j[:], 0.0)
        psj = psjpool.tile([P, 1, 512], F32, tag="psj")

        def warm(n):
            for _ in range(n):
                nc.tensor.matmul(
                    psj[:, 0, 0:512], lhs_j[:], rhs_j[:],
                    start=True, stop=True,
                    perf_mode=mybir.MatmulPerfMode.DoubleRow,
                )

        bb = bpool.tile([P, VSH], F32, tag="bias")

        def stage_bias(g):
            v0 = g * GV
            nc.sync.dma_start(
                bb[:, v0:v0 + GV], bias_d[v0:v0 + GV].partition_broadcast(P))

        h_tiles = [None] * NCH

        def stage_h(c):
            # piece order s-outer/kp-inner so early token blocks complete
            # (and unblock their matmuls) before the whole chunk lands
            hc = hpool.tile([P, KP2, 2, CHT], FP8, tag="hT")
            kps = list(reversed(range(KP2))) if KP_REV_H else list(range(KP2))
            for s in range(CHT // HSP):
                for kp in kps:
                    t0 = c * CHT + s * HSP
                    st = hstage.tile([P, 2, HSP], F32, tag="hstage")
                    _lab(nc.sync.dma_start(st[:], h_r2[kp][:, :, t0:t0 + HSP]),
                         f"dma_h c{c} s{s} kp{kp}")
                    _lab(nc.gpsimd.tensor_copy(
                        hc[:, kp, :, s * HSP:(s + 1) * HSP], st[:]),
                         f"cast_h c{c} s{s} kp{kp}")
            h_tiles[c] = hc

        wv = wpool.tile([P, KP2, 2, VSH], FP8, tag="w")

        def stage_w(g):
            v0 = g * GV
            for kp in (reversed(range(KP2)) if KP_REV_W else range(KP2)):
                ws = wstage.tile([P, 2, GV], F32, tag="wstage")
                _lab(nc.sync.dma_start(ws[:], W_r2[kp][:, :, v0:v0 + GV]),
                     f"dma_w g{g} kp{kp}")
                _lab(nc.gpsimd.tensor_scalar_mul(
                    wv[:, kp, :, v0:v0 + GV], ws[:], W_SCALE),
                     f"cast_w g{g} kp{kp}")

        def compute(c, mm, g):
            m = c * MBC + mm
            pt = pspool.tile([P, BPG, 512], F32, tag="ps")
            lhsT = h_tiles[c][:, :, :, mm * P:(mm + 1) * P]
            # kp descending: the first matmul issued depends on the LAST
            # W/h piece to arrive, so a tile's 16 matmuls run as one warm
            # PE stretch instead of trickling at cold pstate per piece
            kpo = list(reversed(range(KP2))) if KP_REV_MM else list(range(KP2))
            for ki, kp in enumerate(kpo):
                for bk in range(BPG):
                    _lab(nc.tensor.matmul(
                        pt[:, bk, 0:BANK], lhsT[:, kp],
                        wv[:, kp, :, g * GV + bk * BANK:g * GV + (bk + 1) * BANK],
                        start=(ki == 0), stop=(ki == KP2 - 1),
                        perf_mode=mybir.MatmulPerfMode.DoubleRow,
                    ), f"mm c{c} m{mm} g{g} kp{kp} bk{bk}")
            psl = pt[:, 0:BPG, 0:BANK]
            bbv = bb[:, g * GV:(g + 1) * GV].rearrange("p (b c) -> p b c", c=BANK)
            _lab(nc.vector.scalar_tensor_tensor(
                psl, psl, 1.0 / W_SCALE, bbv, op0=Alu.mult, op1=Alu.add),
                 f"bias c{c} m{mm} g{g}")
            et = ejunk.tile([P, BPG, BANK], F32, tag="ejunk")
            col = m * NG + g
            _lab(nc.scalar.activation(
                et[:], psl, Act.Exp, accum_out=s_cols[:, col:col + 1]),
                 f"exp c{c} m{mm} g{g}")

        def rowdot(r):
            # exact f32 target logit for token block r of this core's slice
            for hh in range(2):
                hg = gpool.tile([P, DHALF], F32, tag="hg")
                nc.sync.dma_start(
                    hg[:], hn_d[r * P:(r + 1) * P, hh * DHALF:(hh + 1) * DHALF])
                wgt = gpool.tile([P, DHALF], F32, tag="wgt")
                nc.sync.dma_start(
                    wgt[:], wg_d[r * P:(r + 1) * P, hh * DHALF:(hh + 1) * DHALF])
                dj = djunk.tile([P, DHALF], F32, tag="djunk")
                nc.vector.tensor_tensor_reduce(
                    dj[:], hg[:], wgt[:], 1.0, 0.0, op0=Alu.mult, op1=Alu.add,
                    accum_out=t_cols[:, r * 2 + hh:r * 2 + hh + 1])

        # -- prologue: interleave W groups, bias slices and h chunks on the
        # DMA queue; traverse compute in the same order the data arrives so
        # the in-order PE stream never waits on a far-future transfer --
        stage_w(0)
        stage_bias(0)
        stage_h(0)
        stage_h(1)
        stage_w(1)
        stage_bias(1)
        stage_h(2)
        stage_w(2)
        stage_bias(2)
        stage_w(3)
        stage_bias(3)

        warm(N_WARM0)
        for ui, (c, g) in enumerate((
            (0, 0), (1, 0), (0, 1), (1, 1), (2, 0), (2, 1),
            (0, 2), (1, 2), (2, 2), (0, 3), (1, 3), (2, 3),
        )):
            for mm in range(MBC):
                compute(c, mm, g)
            warm(GAP_DUMMIES.get(ui, 0))

        # steady state: prefetch two chunks ahead, compute chunk c
        stage_h(3)
        if PREFETCH2:
            stage_h(4)
        for c in range(3, NCH):
            if (c + 2 < NCH) if PREFETCH2 else (c + 1 < NCH and c >= 3):
                stage_h(c + 2 if PREFETCH2 else c + 1)
            for mm in range(MBC):
                for g in range(NG):
                    compute(c, mm, g)
            # spread the 8 exact-tgt rowdots over mid-stream chunks
            if 3 <= c <= 6:
                rowdot(2 * (c - 3))
                rowdot(2 * (c - 3) + 1)

        nc.sync.dma_start(s_out[:], s_cols[:])
        nc.sync.dma_start(t_out[:], t_cols[:])

    nc.compile()
    return nc


_NC_CACHE = {}


def _get_program():
    if "v" not in _NC_CACHE:
        _NC_CACHE["v"] = _build()
    return _NC_CACHE["v"]


def kernel(hidden_states, head_weight, head_bias, loss_weight, labels,
           chunk_size=None, **_unused):
    hidden = np.asarray(hidden_states, dtype=np.float32)
    W = np.asarray(head_weight, dtype=np.float32)
    bias = np.asarray(head_bias, dtype=np.float32)
    lw = np.asarray(loss_weight, dtype=np.float32)
    labels = np.asarray(labels).astype(np.int64)

    assert hidden.shape == (N_TOK, D) and W.shape == (V, D)

    nc = _get_program()
    Wt = np.ascontiguousarray(W.T)                 # [D, V]
    ht = np.ascontiguousarray(hidden.T)            # [D, N]
    Wg = W[labels]                                 # gathered rows [N, D]
    in_maps = []
    for c in range(N_CORES):
        vsl = slice(c * VSH, (c + 1) * VSH)
        tsl = slice(c * TC, (c + 1) * TC)
        in_maps.append(dict(
            h=ht,
            W=np.ascontiguousarray(Wt[:, vsl]),
            bias=np.ascontiguousarray(bias[vsl]),
            hn=np.ascontiguousarray(hidden[tsl]),
            wg=np.ascontiguousarray(Wg[tsl]),
        ))
    res = run_bass_kernel_spmd(nc, in_maps, list(range(N_CORES)))

    # unshard + host-side scalar combine (the "all_reduce" of the hint):
    # sum the 8 per-core vocab-shard partials of sum_v exp(logit) per token
    s = np.zeros(N_TOK, dtype=np.float64)
    for r in res.results:
        sc = r["s_out"].astype(np.float64).reshape(P, MBT, NG).sum(axis=2)
        s += sc.T.reshape(N_TOK)
    # exact f32 target dot h . W[label] (+ bias) per token
    tgt = np.concatenate([
        r["t_out"].astype(np.float64).reshape(P, TC // P, 2).sum(axis=2)
        .T.reshape(TC)
        for r in res.results])
    tgt = tgt + bias[labels].astype(np.float64)
    lse = np.log(s)
    nll = lse - tgt
    w64 = lw.astype(np.float64)
    loss = (w64 * nll).sum() / max(w64.sum(), 1.0)
    return np.float32(loss)


# revision 18
# speedup vs baseline: 1.0200x; 1.0063x over previous
"""Fused linear + cross-entropy loss (BaseChunkLoss) on 8 trn2 NeuronCores.

Strategy (per the sharding hint: tensor-parallel over vocab):
  - head_weight is sharded 8 ways over the vocab dim: each core handles the
    FULL 8192 tokens x a 4000-entry vocab slice and produces the partial
    sum_{v in shard} exp(logit[t, v]) for every token.  The cross-device
    logsumexp reduction (sum of the 8 partials, then log) plus the weighted
    mean happen on host, standing in for the wrapper's all_reduce.
  - This puts each core's HBM traffic at ~117 MB (full hidden 67 MB + W
    slice 33 MB + target-row gather 17 MB) -- under the fp8 PE roofline --
    instead of the 290 MB/core a token-sharded design pays to stream the
    whole 262 MB weight through every core.
  - The W slice is cast to fp8 once and stays SBUF-resident; hidden^T
    streams through in 1024-token chunks, cast to fp8 on the fly.

Device kernel layout: tokens on PSUM partitions, vocab on the free dim.
  stationary lhsT = hidden^T tile [128 d x 2 x 128 tok]   (fp8, DoubleRow)
  moving rhs      = weight^T tile [128 d x 2 x 500 vocab]
  psum [128 tok x 500 vocab] fp32, accumulated over D=2048 in 8 matmuls.
Weights are pre-scaled by 64 on-chip for e4m3 range and descaled during the
bias add.  Per 1000-wide vocab group (2 psum banks, 4 groups in flight):
DVE does (psum/64 + bias) in place, ACT computes exp with a fused free-dim
row-sum accumulator into s_cols.  The target logit is computed exactly in
f32 as a DVE rowdot of the core's 1024-token hidden slice against the
host-gathered W[labels] rows; host adds bias[labels].

Host-side input prep is layout-only (transpose/slice/gather of rows); all
FLOPs over hidden/weights happen on device inside the measured kernel.
"""
import numpy as np
from contextlib import ExitStack

from concourse import bacc, mybir, tile
from concourse.bass_utils import run_bass_kernel_spmd

F32 = mybir.dt.float32
FP8 = mybir.dt.float8e4
Alu = mybir.AluOpType
Act = mybir.ActivationFunctionType

N_CORES = 8
N_TOK = 8192
D = 2048
V = 32000
P = 128

VSH = V // N_CORES      # 4000 vocab entries per core
TC = N_TOK // N_CORES   # 1024 tokens per core (for the exact tgt rowdot)
KP2 = D // (2 * P)      # 8 DoubleRow contraction steps of K=256
BANK = 500              # vocab columns per psum bank (<= 512 fp32)
BPG = 2                 # banks per vocab group
GV = BPG * BANK         # 1000 vocab per group
NG = VSH // GV          # 4 groups
CHT = 1024              # tokens per streamed hidden chunk
NCH = N_TOK // CHT      # 8 chunks
MBC = CHT // P          # 8 token blocks per chunk
MBT = N_TOK // P        # 64 token blocks total
HSP = 512               # tokens per hidden DMA piece
DHALF = D // 2          # rowdot split for SBUF economy

W_SCALE = 64.0          # fp8 weight pre-scale (e4m3 range)

# schedule knobs (tuned empirically against TimelineSim)
KP_REV_MM = False       # issue matmuls kp-descending
KP_REV_W = False        # stage W pieces kp-descending
KP_REV_H = False        # stage h pieces kp-descending
H_BUFS = 3              # hT chunk double-buffer depth
PREFETCH2 = False       # prefetch two h chunks ahead
# PE pstate warmup: dummy matmuls keep the PE "busy" across DMA-wait gaps so
# real matmuls always run at full clock (the cost model halves the clock for
# ~3us after every idle->busy transition).  _WARM_PLAN maps a real-matmul
# ordinal to the number of dummy matmuls issued just before it (auto-tuned
# against TimelineSim by autotune.py; see _apply_warm_plan).
_WARM_PLAN = {}
_MM_KINDS = []          # emission-order record: "d"=dummy, int=real ordinal

_DBG_LABELS = {}


def _lab(inst, label):
    try:
        _DBG_LABELS[inst.name] = label
    except Exception:
        pass
    return inst


def _build():
    nc = bacc.Bacc("TRN2", target_bir_lowering=False, debug=False)
    h_d = nc.declare_dram_parameter("h", [D, N_TOK], F32, isOutput=False)
    W_d = nc.declare_dram_parameter("W", [D, VSH], F32, isOutput=False)
    bias_d = nc.declare_dram_parameter("bias", [VSH], F32, isOutput=False)
    hn_d = nc.declare_dram_parameter("hn", [TC, D], F32, isOutput=False)
    wg_d = nc.declare_dram_parameter("wg", [TC, D], F32, isOutput=False)
    s_out = nc.declare_dram_parameter("s_out", [P, MBT * NG], F32, isOutput=True)
    t_out = nc.declare_dram_parameter("t_out", [P, TC // P * 2], F32, isOutput=True)

    h_r2 = h_d[:].rearrange("(kp j ki) t -> kp ki j t", ki=P, j=2)
    W_r2 = W_d[:].rearrange("(kp j ki) v -> kp ki j v", ki=P, j=2)

    with tile.TileContext(nc) as tc, ExitStack() as ctx:
        wpool = ctx.enter_context(tc.tile_pool(name="w", bufs=1))
        wstage = ctx.enter_context(tc.tile_pool(name="wstage", bufs=2))
        hpool = ctx.enter_context(tc.tile_pool(name="hT", bufs=H_BUFS))
        hstage = ctx.enter_context(tc.tile_pool(name="hstage", bufs=2))
        bpool = ctx.enter_context(tc.tile_pool(name="bias", bufs=1))
        gpool = ctx.enter_context(tc.tile_pool(name="gath", bufs=2))
        djunk = ctx.enter_context(tc.tile_pool(name="djunk", bufs=1))
        ejunk = ctx.enter_context(tc.tile_pool(name="ejunk", bufs=2))
        pspool = ctx.enter_context(tc.tile_pool(name="ps", bufs=4, space="PSUM"))
        acc = ctx.enter_context(tc.tile_pool(name="acc", bufs=1))

        s_cols = acc.tile([P, MBT * NG], F32, tag="scols")
        t_cols = acc.tile([P, TC // P * 2], F32, tag="tcols")

        mm_ord = [0]

        def real_mm(*args, **kwargs):
            _MM_KINDS.append(mm_ord[0])
            mm_ord[0] += 1
            return nc.tensor.matmul(*args, **kwargs)

        bb = bpool.tile([P, VSH], F32, tag="bias")

        def stage_bias(g):
            v0 = g * GV
            nc.sync.dma_start(
                bb[:, v0:v0 + GV], bias_d[v0:v0 + GV].partition_broadcast(P))

        h_tiles = [None] * NCH

        def stage_h(c):
            # piece order s-outer/kp-inner so early token blocks complete
            # (and unblock their matmuls) before the whole chunk lands
            hc = hpool.tile([P, KP2, 2, CHT], FP8, tag="hT")
            kps = list(reversed(range(KP2))) if KP_REV_H else list(range(KP2))
            for s in range(CHT // HSP):
                for kp in kps:
                    t0 = c * CHT + s * HSP
                    st = hstage.tile([P, 2, HSP], F32, tag="hstage")
                    _lab(nc.sync.dma_start(st[:], h_r2[kp][:, :, t0:t0 + HSP]),
                         f"dma_h c{c} s{s} kp{kp}")
                    _lab(nc.gpsimd.tensor_copy(
                        hc[:, kp, :, s * HSP:(s + 1) * HSP], st[:]),
                         f"cast_h c{c} s{s} kp{kp}")
            h_tiles[c] = hc

        wv = wpool.tile([P, KP2, 2, VSH], FP8, tag="w")

        def stage_w(g):
            v0 = g * GV
            for kp in (reversed(range(KP2)) if KP_REV_W else range(KP2)):
                ws = wstage.tile([P, 2, GV], F32, tag="wstage")
                _lab(nc.sync.dma_start(ws[:], W_r2[kp][:, :, v0:v0 + GV]),
                     f"dma_w g{g} kp{kp}")
                _lab(nc.gpsimd.tensor_scalar_mul(
                    wv[:, kp, :, v0:v0 + GV], ws[:], W_SCALE),
                     f"cast_w g{g} kp{kp}")

        def compute(c, mm, g):
            m = c * MBC + mm
            pt = pspool.tile([P, BPG, 512], F32, tag="ps")
            lhsT = h_tiles[c][:, :, :, mm * P:(mm + 1) * P]
            # kp descending: the first matmul issued depends on the LAST
            # W/h piece to arrive, so a tile's 16 matmuls run as one warm
            # PE stretch instead of trickling at cold pstate per piece
            kpo = list(reversed(range(KP2))) if KP_REV_MM else list(range(KP2))
            for ki, kp in enumerate(kpo):
                for bk in range(BPG):
                    _lab(real_mm(
                        pt[:, bk, 0:BANK], lhsT[:, kp],
                        wv[:, kp, :, g * GV + bk * BANK:g * GV + (bk + 1) * BANK],
                        start=(ki == 0), stop=(ki == KP2 - 1),
                        perf_mode=mybir.MatmulPerfMode.DoubleRow,
                    ), f"mm c{c} m{mm} g{g} kp{kp} bk{bk}")
            psl = pt[:, 0:BPG, 0:BANK]
            bbv = bb[:, g * GV:(g + 1) * GV].rearrange("p (b c) -> p b c", c=BANK)
            _lab(nc.vector.scalar_tensor_tensor(
                psl, psl, 1.0 / W_SCALE, bbv, op0=Alu.mult, op1=Alu.add),
                 f"bias c{c} m{mm} g{g}")
            et = ejunk.tile([P, BPG, BANK], F32, tag="ejunk")
            col = m * NG + g
            _lab(nc.scalar.activation(
                et[:], psl, Act.Exp, accum_out=s_cols[:, col:col + 1]),
                 f"exp c{c} m{mm} g{g}")

        def rowdot(r):
            # exact f32 target logit for token block r of this core's slice
            for hh in range(2):
                hg = gpool.tile([P, DHALF], F32, tag="hg")
                nc.sync.dma_start(
                    hg[:], hn_d[r * P:(r + 1) * P, hh * DHALF:(hh + 1) * DHALF])
                wgt = gpool.tile([P, DHALF], F32, tag="wgt")
                nc.sync.dma_start(
                    wgt[:], wg_d[r * P:(r + 1) * P, hh * DHALF:(hh + 1) * DHALF])
                dj = djunk.tile([P, DHALF], F32, tag="djunk")
                nc.vector.tensor_tensor_reduce(
                    dj[:], hg[:], wgt[:], 1.0, 0.0, op0=Alu.mult, op1=Alu.add,
                    accum_out=t_cols[:, r * 2 + hh:r * 2 + hh + 1])

        # -- prologue: interleave W groups, bias slices and h chunks on the
        # DMA queue; traverse compute in the same order the data arrives so
        # the in-order PE stream never waits on a far-future transfer --
        stage_w(0)
        stage_bias(0)
        stage_h(0)
        stage_h(1)
        stage_w(1)
        stage_bias(1)
        stage_h(2)
        stage_w(2)
        stage_bias(2)
        stage_w(3)
        stage_bias(3)

        for c, g in (
            (0, 0), (1, 0), (0, 1), (1, 1), (2, 0), (2, 1),
            (0, 2), (1, 2), (2, 2), (0, 3), (1, 3), (2, 3),
        ):
            for mm in range(MBC):
                compute(c, mm, g)

        # steady state: prefetch two chunks ahead, compute chunk c
        stage_h(3)
        if PREFETCH2:
            stage_h(4)
        for c in range(3, NCH):
            if (c + 2 < NCH) if PREFETCH2 else (c + 1 < NCH and c >= 3):
                stage_h(c + 2 if PREFETCH2 else c + 1)
            for mm in range(MBC):
                for g in range(NG):
                    compute(c, mm, g)
            # spread the 8 exact-tgt rowdots over mid-stream chunks
            if 3 <= c <= 6:
                rowdot(2 * (c - 3))
                rowdot(2 * (c - 3) + 1)

        nc.sync.dma_start(s_out[:], s_cols[:])
        nc.sync.dma_start(t_out[:], t_cols[:])

    nc.compile()
    return nc


_NC_CACHE = {}


def _get_program():
    if "v" not in _NC_CACHE:
        _NC_CACHE["v"] = _build()
    return _NC_CACHE["v"]


def kernel(hidden_states, head_weight, head_bias, loss_weight, labels,
           chunk_size=None, **_unused):
    hidden = np.asarray(hidden_states, dtype=np.float32)
    W = np.asarray(head_weight, dtype=np.float32)
    bias = np.asarray(head_bias, dtype=np.float32)
    lw = np.asarray(loss_weight, dtype=np.float32)
    labels = np.asarray(labels).astype(np.int64)

    assert hidden.shape == (N_TOK, D) and W.shape == (V, D)

    nc = _get_program()
    Wt = np.ascontiguousarray(W.T)                 # [D, V]
    ht = np.ascontiguousarray(hidden.T)            # [D, N]
    Wg = W[labels]                                 # gathered rows [N, D]
    in_maps = []
    for c in range(N_CORES):
        vsl = slice(c * VSH, (c + 1) * VSH)
        tsl = slice(c * TC, (c + 1) * TC)
        in_maps.append(dict(
            h=ht,
            W=np.ascontiguousarray(Wt[:, vsl]),
            bias=np.ascontiguousarray(bias[vsl]),
            hn=np.ascontiguousarray(hidden[tsl]),
            wg=np.ascontiguousarray(Wg[tsl]),
        ))
    res = run_bass_kernel_spmd(nc, in_maps, list(range(N_CORES)))

    # unshard + host-side scalar combine (the "all_reduce" of the hint):
    # sum the 8 per-core vocab-shard partials of sum_v exp(logit) per token
    s = np.zeros(N_TOK, dtype=np.float64)
    for r in res.results:
        sc = r["s_out"].astype(np.float64).reshape(P, MBT, NG).sum(axis=2)
        s += sc.T.reshape(N_TOK)
    # exact f32 target dot h . W[label] (+ bias) per token
    tgt = np.concatenate([
        r["t_out"].astype(np.float64).reshape(P, TC // P, 2).sum(axis=2)
        .T.reshape(TC)
        for r in res.results])
    tgt = tgt + bias[labels].astype(np.float64)
    lse = np.log(s)
    nll = lse - tgt
    w64 = lw.astype(np.float64)
    loss = (w64 * nll).sum() / max(w64.sum(), 1.0)
    return np.float32(loss)


# revision 29
# speedup vs baseline: 1.1071x; 1.0854x over previous
"""Fused linear + cross-entropy loss (BaseChunkLoss) on 8 trn2 NeuronCores.

Strategy (per the sharding hint: tensor-parallel over vocab):
  - head_weight is sharded 8 ways over the vocab dim: each core handles the
    FULL 8192 tokens x a 4000-entry vocab slice and produces the partial
    sum_{v in shard} exp(logit[t, v]) for every token.  The cross-device
    logsumexp reduction (sum of the 8 partials, then log) plus the weighted
    mean happen on host, standing in for the wrapper's all_reduce.
  - This puts each core's HBM traffic at ~117 MB (full hidden 67 MB + W
    slice 33 MB + target-row gather 17 MB) -- under the fp8 PE roofline --
    instead of the 290 MB/core a token-sharded design pays to stream the
    whole 262 MB weight through every core.
  - The W slice is cast to fp8 once and stays SBUF-resident; hidden^T
    streams through in 1024-token chunks, cast to fp8 on the fly.

Device kernel layout: tokens on PSUM partitions, vocab on the free dim.
  stationary lhsT = hidden^T tile [128 d x 2 x 128 tok]   (fp8, DoubleRow)
  moving rhs      = weight^T tile [128 d x 2 x 500 vocab]
  psum [128 tok x 500 vocab] fp32, accumulated over D=2048 in 8 matmuls.
Weights are pre-scaled by 64 on-chip for e4m3 range and descaled during the
bias add.  Per 1000-wide vocab group (2 psum banks, 4 groups in flight):
DVE does (psum/64 + bias) in place, ACT computes exp with a fused free-dim
row-sum accumulator into s_cols.  The target logit is computed exactly in
f32 as a DVE rowdot of the core's 1024-token hidden slice against the
host-gathered W[labels] rows; host adds bias[labels].

Host-side input prep is layout-only (transpose/slice/gather of rows); all
FLOPs over hidden/weights happen on device inside the measured kernel.
"""
import numpy as np
from contextlib import ExitStack

from concourse import bacc, mybir, tile
from concourse.bass_utils import run_bass_kernel_spmd

F32 = mybir.dt.float32
FP8 = mybir.dt.float8e4
Alu = mybir.AluOpType
Act = mybir.ActivationFunctionType

N_CORES = 8
N_TOK = 8192
D = 2048
V = 32000
P = 128

VSH = V // N_CORES      # 4000 vocab entries per core
TC = N_TOK // N_CORES   # 1024 tokens per core (for the exact tgt rowdot)
KP2 = D // (2 * P)      # 8 DoubleRow contraction steps of K=256
BANK = 500              # vocab columns per psum bank (<= 512 fp32)
BPG = 2                 # banks per vocab group
GV = BPG * BANK         # 1000 vocab per group
NG = VSH // GV          # 4 groups
CHT = 1024              # tokens per streamed hidden chunk
NCH = N_TOK // CHT      # 8 chunks
MBC = CHT // P          # 8 token blocks per chunk
MBT = N_TOK // P        # 64 token blocks total
HSP = 512               # tokens per hidden DMA piece
DHALF = D // 2          # rowdot split for SBUF economy

W_SCALE = 64.0          # fp8 weight pre-scale (e4m3 range)

# schedule knobs (tuned empirically against TimelineSim)
KP_REV_MM = False       # issue matmuls kp-descending
KP_REV_W = False        # stage W pieces kp-descending
KP_REV_H = False        # stage h pieces kp-descending
H_BUFS = 3              # hT chunk double-buffer depth
PREFETCH2 = False       # prefetch two h chunks ahead
# PE pstate warmup: dummy matmuls keep the PE "busy" across DMA-wait gaps so
# real matmuls always run at full clock (the cost model halves the clock for
# ~3us after every idle->busy transition).  _WARM_PLAN maps a real-matmul
# ordinal to the number of dummy matmuls issued just before it (auto-tuned
# against TimelineSim by autotune.py; see _apply_warm_plan).
_WARM_PLAN = {}
_MM_KINDS = []          # emission-order record: "d"=dummy, int=real ordinal

_DBG_LABELS = {}


def _lab(inst, label):
    try:
        _DBG_LABELS[inst.name] = label
    except Exception:
        pass
    return inst


def _build():
    nc = bacc.Bacc("TRN2", target_bir_lowering=False, debug=False)
    h_d = nc.declare_dram_parameter("h", [D, N_TOK], F32, isOutput=False)
    W_d = nc.declare_dram_parameter("W", [D, VSH], F32, isOutput=False)
    bias_d = nc.declare_dram_parameter("bias", [VSH], F32, isOutput=False)
    hn_d = nc.declare_dram_parameter("hn", [TC, D], F32, isOutput=False)
    wg_d = nc.declare_dram_parameter("wg", [TC, D], F32, isOutput=False)
    s_out = nc.declare_dram_parameter("s_out", [P, MBT * NG], F32, isOutput=True)
    t_out = nc.declare_dram_parameter("t_out", [P, TC // P * 2], F32, isOutput=True)

    h_r2 = h_d[:].rearrange("(kp j ki) t -> kp ki j t", ki=P, j=2)
    W_r2 = W_d[:].rearrange("(kp j ki) v -> kp ki j v", ki=P, j=2)

    with tile.TileContext(nc) as tc, ExitStack() as ctx:
        wpool = ctx.enter_context(tc.tile_pool(name="w", bufs=1))
        wstage = ctx.enter_context(tc.tile_pool(name="wstage", bufs=4))
        hpool = ctx.enter_context(tc.tile_pool(name="hT", bufs=H_BUFS))
        hstage = ctx.enter_context(tc.tile_pool(name="hstage", bufs=4))
        bpool = ctx.enter_context(tc.tile_pool(name="bias", bufs=1))
        gpool = ctx.enter_context(tc.tile_pool(name="gath", bufs=2))
        djunk = ctx.enter_context(tc.tile_pool(name="djunk", bufs=1))
        ejunk = ctx.enter_context(tc.tile_pool(name="ejunk", bufs=2))
        pspool = ctx.enter_context(tc.tile_pool(name="ps", bufs=4, space="PSUM"))
        acc = ctx.enter_context(tc.tile_pool(name="acc", bufs=1))

        s_cols = acc.tile([P, MBT * NG], F32, tag="scols")
        t_cols = acc.tile([P, TC // P * 2], F32, tag="tcols")

        mm_ord = [0]

        def real_mm(*args, **kwargs):
            _MM_KINDS.append(mm_ord[0])
            mm_ord[0] += 1
            return nc.tensor.matmul(*args, **kwargs)

        bb = bpool.tile([P, VSH], F32, tag="bias")

        def stage_bias(g):
            v0 = g * GV
            nc.sync.dma_start(
                bb[:, v0:v0 + GV], bias_d[v0:v0 + GV].partition_broadcast(P))

        h_tiles = [None] * NCH

        def stage_h_half(c, s):
            hc = h_tiles[c]
            kps = list(reversed(range(KP2))) if KP_REV_H else list(range(KP2))
            for kp in kps:
                t0 = c * CHT + s * HSP
                st = hstage.tile([P, 2, HSP], F32, tag="hstage")
                _lab(nc.sync.dma_start(st[:], h_r2[kp][:, :, t0:t0 + HSP]),
                     f"dma_h c{c} s{s} kp{kp}")
                _lab(nc.gpsimd.tensor_copy(
                    hc[:, kp, :, s * HSP:(s + 1) * HSP], st[:]),
                     f"cast_h c{c} s{s} kp{kp}")

        def stage_h(c):
            # piece order s-outer/kp-inner so early token blocks complete
            # (and unblock their matmuls) before the whole chunk lands
            hc = hpool.tile([P, KP2, 2, CHT], FP8, tag="hT")
            h_tiles[c] = hc
            for s in range(CHT // HSP):
                stage_h_half(c, s)

        wv = wpool.tile([P, KP2, 2, VSH], FP8, tag="w")

        def stage_w_piece(g, kp):
            v0 = g * GV
            ws = wstage.tile([P, 2, GV], F32, tag="wstage")
            _lab(nc.sync.dma_start(ws[:], W_r2[kp][:, :, v0:v0 + GV]),
                 f"dma_w g{g} kp{kp}")
            _lab(nc.gpsimd.tensor_scalar_mul(
                wv[:, kp, :, v0:v0 + GV], ws[:], W_SCALE),
                 f"cast_w g{g} kp{kp}")

        def stage_w(g):
            for kp in (reversed(range(KP2)) if KP_REV_W else range(KP2)):
                stage_w_piece(g, kp)

        def compute(c, mm, g):
            m = c * MBC + mm
            pt = pspool.tile([P, BPG, 512], F32, tag="ps")
            lhsT = h_tiles[c][:, :, :, mm * P:(mm + 1) * P]
            # kp descending: the first matmul issued depends on the LAST
            # W/h piece to arrive, so a tile's 16 matmuls run as one warm
            # PE stretch instead of trickling at cold pstate per piece
            kpo = list(reversed(range(KP2))) if KP_REV_MM else list(range(KP2))
            for ki, kp in enumerate(kpo):
                for bk in range(BPG):
                    _lab(real_mm(
                        pt[:, bk, 0:BANK], lhsT[:, kp],
                        wv[:, kp, :, g * GV + bk * BANK:g * GV + (bk + 1) * BANK],
                        start=(ki == 0), stop=(ki == KP2 - 1),
                        perf_mode=mybir.MatmulPerfMode.DoubleRow,
                    ), f"mm c{c} m{mm} g{g} kp{kp} bk{bk}")
            psl = pt[:, 0:BPG, 0:BANK]
            bbv = bb[:, g * GV:(g + 1) * GV].rearrange("p (b c) -> p b c", c=BANK)
            _lab(nc.vector.scalar_tensor_tensor(
                psl, psl, 1.0 / W_SCALE, bbv, op0=Alu.mult, op1=Alu.add),
                 f"bias c{c} m{mm} g{g}")
            et = ejunk.tile([P, BPG, BANK], F32, tag="ejunk")
            col = m * NG + g
            _lab(nc.scalar.activation(
                et[:], psl, Act.Exp, accum_out=s_cols[:, col:col + 1]),
                 f"exp c{c} m{mm} g{g}")

        def compute_set(c, g, mms):
            # kp-major across <=4 tiles: each arriving W k-piece unlocks
            # matmuls for every tile in the set (in-order PE trickles deeper)
            pts = []
            for _ in mms:
                pt = pspool.tile([P, BPG, 512], F32, tag="ps")
                pts.append(pt)
            kpo = list(reversed(range(KP2))) if KP_REV_MM else list(range(KP2))
            for ki, kp in enumerate(kpo):
                for i, mm in enumerate(mms):
                    lhsT = h_tiles[c][:, :, :, mm * P:(mm + 1) * P]
                    for bk in range(BPG):
                        _lab(real_mm(
                            pts[i][:, bk, 0:BANK], lhsT[:, kp],
                            wv[:, kp, :, g * GV + bk * BANK:g * GV + (bk + 1) * BANK],
                            start=(ki == 0), stop=(ki == KP2 - 1),
                            perf_mode=mybir.MatmulPerfMode.DoubleRow,
                        ), f"mm c{c} m{mm} g{g} kp{kp} bk{bk}")
            for i, mm in enumerate(mms):
                m = c * MBC + mm
                pt = pts[i]
                psl = pt[:, 0:BPG, 0:BANK]
                bbv = bb[:, g * GV:(g + 1) * GV].rearrange(
                    "p (b c) -> p b c", c=BANK)
                _lab(nc.vector.scalar_tensor_tensor(
                    psl, psl, 1.0 / W_SCALE, bbv, op0=Alu.mult, op1=Alu.add),
                     f"bias c{c} m{mm} g{g}")
                et = ejunk.tile([P, BPG, BANK], F32, tag="ejunk")
                col = m * NG + g
                _lab(nc.scalar.activation(
                    et[:], psl, Act.Exp, accum_out=s_cols[:, col:col + 1]),
                     f"exp c{c} m{mm} g{g}")

        def rowdot(r):
            # exact f32 target logit for token block r of this core's slice
            for hh in range(2):
                hg = gpool.tile([P, DHALF], F32, tag="hg")
                nc.sync.dma_start(
                    hg[:], hn_d[r * P:(r + 1) * P, hh * DHALF:(hh + 1) * DHALF])
                wgt = gpool.tile([P, DHALF], F32, tag="wgt")
                nc.sync.dma_start(
                    wgt[:], wg_d[r * P:(r + 1) * P, hh * DHALF:(hh + 1) * DHALF])
                dj = djunk.tile([P, DHALF], F32, tag="djunk")
                nc.vector.tensor_tensor_reduce(
                    dj[:], hg[:], wgt[:], 1.0, 0.0, op0=Alu.mult, op1=Alu.add,
                    accum_out=t_cols[:, r * 2 + hh:r * 2 + hh + 1])

        # -- prologue: interleave W groups, bias slices and h chunks on the
        # DMA queue; traverse compute in the same order the data arrives so
        # the in-order PE stream never waits on a far-future transfer --
        hc = hpool.tile([P, KP2, 2, CHT], FP8, tag="hT")
        h_tiles[0] = hc
        stage_h_half(0, 0)
        stage_w(0)
        stage_bias(0)
        stage_h_half(0, 1)
        stage_h(1)
        stage_w(1)
        stage_bias(1)
        stage_h(2)
        stage_w(2)
        stage_bias(2)
        stage_w(3)
        stage_bias(3)

        for c, g in (
            (0, 0), (1, 0), (0, 1), (1, 1), (2, 0), (2, 1),
            (0, 2), (1, 2), (2, 2), (0, 3), (1, 3), (2, 3),
        ):
            for mm in range(MBC):
                compute(c, mm, g)

        # steady state: prefetch two chunks ahead, compute chunk c
        stage_h(3)
        if PREFETCH2:
            stage_h(4)
        for c in range(3, NCH):
            if (c + 2 < NCH) if PREFETCH2 else (c + 1 < NCH and c >= 3):
                stage_h(c + 2 if PREFETCH2 else c + 1)
            for mm in range(MBC):
                for g in range(NG):
                    compute(c, mm, g)
            # spread the 8 exact-tgt rowdots over mid-stream chunks
            if 3 <= c <= 6:
                rowdot(2 * (c - 3))
                rowdot(2 * (c - 3) + 1)
            if c == 6:
                nc.sync.dma_start(t_out[:], t_cols[:])
                # chunks 0-6 columns of s_cols are final once chunk 6 drains
                nc.sync.dma_start(
                    s_out[0:P, 0:7 * MBC * NG], s_cols[:, 0:7 * MBC * NG])

        nc.sync.dma_start(
            s_out[0:P, 7 * MBC * NG:MBT * NG], s_cols[:, 7 * MBC * NG:MBT * NG])

    nc.compile()
    return nc


_NC_CACHE = {}


def _get_program():
    if "v" not in _NC_CACHE:
        _NC_CACHE["v"] = _build()
    return _NC_CACHE["v"]


def kernel(hidden_states, head_weight, head_bias, loss_weight, labels,
           chunk_size=None, **_unused):
    hidden = np.asarray(hidden_states, dtype=np.float32)
    W = np.asarray(head_weight, dtype=np.float32)
    bias = np.asarray(head_bias, dtype=np.float32)
    lw = np.asarray(loss_weight, dtype=np.float32)
    labels = np.asarray(labels).astype(np.int64)

    assert hidden.shape == (N_TOK, D) and W.shape == (V, D)

    nc = _get_program()
    Wt = np.ascontiguousarray(W.T)                 # [D, V]
    ht = np.ascontiguousarray(hidden.T)            # [D, N]
    Wg = W[labels]                                 # gathered rows [N, D]
    in_maps = []
    for c in range(N_CORES):
        vsl = slice(c * VSH, (c + 1) * VSH)
        tsl = slice(c * TC, (c + 1) * TC)
        in_maps.append(dict(
            h=ht,
            W=np.ascontiguousarray(Wt[:, vsl]),
            bias=np.ascontiguousarray(bias[vsl]),
            hn=np.ascontiguousarray(hidden[tsl]),
            wg=np.ascontiguousarray(Wg[tsl]),
        ))
    res = run_bass_kernel_spmd(nc, in_maps, list(range(N_CORES)))

    # unshard + host-side scalar combine (the "all_reduce" of the hint):
    # sum the 8 per-core vocab-shard partials of sum_v exp(logit) per token
    s = np.zeros(N_TOK, dtype=np.float64)
    for r in res.results:
        sc = r["s_out"].astype(np.float64).reshape(P, MBT, NG).sum(axis=2)
        s += sc.T.reshape(N_TOK)
    # exact f32 target dot h . W[label] (+ bias) per token
    tgt = np.concatenate([
        r["t_out"].astype(np.float64).reshape(P, TC // P, 2).sum(axis=2)
        .T.reshape(TC)
        for r in res.results])
    tgt = tgt + bias[labels].astype(np.float64)
    lse = np.log(s)
    nll = lse - tgt
    w64 = lw.astype(np.float64)
    loss = (w64 * nll).sum() / max(w64.sum(), 1.0)
    return np.float32(loss)
